# revision 14
# baseline (speedup 1.0000x reference)
"""BiasedMultiHeadAttention Trainium2 kernel (fp8 DoubleRow pipeline).

Sharding: 8 cores = (batch b, query-half qh). Each core computes the full
pipeline for its 512 query rows of batch b (K/V projections for the batch
are duplicated across the 2 cores sharing it). No collectives.

Device layout trick: per-core x rows are host-rolled so the core's query
block is always rows 0..511 -> one SPMD program for all 8 cores; bias/mask
are rolled consistently (softmax sum order irrelevant).

Perf structure:
  - All projections + AV run as fp8e4 DoubleRow matmuls (2x PE rate).
    Weights are host-scaled by a power of two into fp8 range; the inverse
    scale is folded into the PSUM->SBUF copy (tensor_scalar mul) or the
    final residual add, so numerics stay at natural scale in bf16/fp32.
  - QK stays bf16 (output-bound: free-dim cycles dominate either way).
  - Attention bias-add happens in-place in PSUM, split across Vector
    (head A) and GpSimd (head B); Scalar engine does only the exp.
  - exp computed with a constant -SHIFT bias so fp8 'at' can't overflow;
    the shift cancels exactly in the rowsum normalization.
  - Emission interleaves attention head-pairs 0..2 with the remaining
    projection matmuls so the PE never drains.

Math folding (host, exact):
  xn_aff = ln(x)*g + b folded into weights:  w_eff[i,o] = w[o,i]*ln_g[i]
  b_eff[o] = (w @ ln_b + b)[o];  Q additionally scaled by SCALE/gate_h and
  exp computed as exp(gate_h * s + key_mask - SHIFT) via ACT operands.
"""

import math

import numpy as np
import ml_dtypes

import concourse.bass as bass
import concourse.tile as tile
import concourse.mybir as mybir
from concourse import bacc
from concourse.bass_utils import run_bass_kernel_spmd
from concourse.masks import make_identity

B, L, E, H = 4, 1024, 1024, 16
D = E // H
SCALE = D**-0.5
EPS = 1e-5
NCORES = 8
QL = 512  # query rows per core
PT = 128  # partitions
NL = L // PT  # 8 l-chunks
NE = E // PT  # 8 e-chunks
HP = H // 2  # 8 head pairs
CP = NL // 2  # 4 key-chunk pairs
SHIFT = 1.5  # exp(x - SHIFT); cancels in normalization

F32 = mybir.dt.float32
BF16 = mybir.dt.bfloat16
FP8 = mybir.dt.float8e4
I32 = mybir.dt.int32
BF_NP = ml_dtypes.bfloat16
FP8_NP = ml_dtypes.float8_e4m3
DR = mybir.MatmulPerfMode.DoubleRow

LAST_RESULT = None  # BassKernelResults of the most recent run (for test.py)


def _build_nc(gates, inv_scales, use_pbias, use_mask):
    """Build the single-core Bass program (same NEFF for all 8 cores).

    gates: 16 python floats (exp scale immediates)
    inv_scales: dict name -> float, inverse of the host fp8 weight scaling
    use_pbias: 4 bools - include projection-bias rank-1 matmuls for q,k,v,o
    use_mask: include key/query mask handling
    """
    nc = bacc.Bacc("TRN2", target_bir_lowering=False, debug=False)

    x_d = nc.dram_tensor("xc", [PT, NL, L], F32, kind="ExternalInput")
    bias_d = nc.dram_tensor("biasc", [HP, CP, PT, 2, 2, QL], FP8,
                            kind="ExternalInput")
    wq_d = nc.dram_tensor("wqt", [PT, NE, E], FP8, kind="ExternalInput")
    wk_d = nc.dram_tensor("wkt", [PT, NE, E], FP8, kind="ExternalInput")
    wv_d = nc.dram_tensor("wvt", [PT, NE, E], FP8, kind="ExternalInput")
    wo_d = nc.dram_tensor("wot", [PT, NE, E], FP8, kind="ExternalInput")
    zz_d = nc.dram_tensor("zz", [1, NE * L], BF16, kind="ExternalInput")
    pb_d = {}
    for name, use in zip("qkvo", use_pbias):
        if use:
            pb_d[name] = nc.dram_tensor(f"b{name}e", [1, E], BF16,
                                        kind="ExternalInput")
    if use_mask:
        km_d = nc.dram_tensor("kmc", [PT, NL], F32, kind="ExternalInput")
        mq_d = nc.dram_tensor("mqc", [1, 2 * QL], F32, kind="ExternalInput")
    y_d = nc.dram_tensor("yc", [QL, E], F32, kind="ExternalOutput")

    iq, ik, iv, io = (inv_scales[n] for n in "qkvo")
    same_gate = len(set(gates)) == 1

    with tile.TileContext(nc) as tc:
        with (
            tc.tile_pool(name="persist", bufs=1) as pp,
            tc.tile_pool(name="consts", bufs=1) as cp,
        ):
            # ---- constants ----
            ident = cp.tile([PT, PT], BF16)
            make_identity(nc, ident)
            ones_row = cp.tile([1, L], BF16)
            nc.vector.memset(ones_row, 1.0)
            eps_t = cp.tile([PT, 1], F32)
            nc.vector.memset(eps_t, EPS)
            shift_t = cp.tile([PT, 1], F32)
            nc.vector.memset(shift_t, -SHIFT)
            if use_mask:
                km_sb = cp.tile([PT, NL], F32)
                nc.sync.dma_start(km_sb, km_d[:, :])
                mqb = cp.tile([64, 2 * QL], F32)
                nc.gpsimd.dma_start(mqb,
                                    mq_d[0:1, :].partition_broadcast(64))

            # ---- resident tensors ----
            x_sb = pp.tile([PT, NL, L], F32)
            for lt in range(NL):
                nc.sync.dma_start(x_sb[:, lt, :], x_d[:, lt, :])
            wq_sb = pp.tile([PT, NE, E], FP8)
            nc.sync.dma_start(wq_sb, wq_d[:, :, :])
            wk_sb = pp.tile([PT, NE, E], FP8)
            nc.sync.dma_start(wk_sb, wk_d[:, :, :])
            wv_sb = pp.tile([PT, NE, E], FP8)
            nc.sync.dma_start(wv_sb, wv_d[:, :, :])
            wo_sb = pp.tile([PT, NE, E], FP8)
            nc.sync.dma_start(wo_sb, wo_d[:, :, :])
            xnT = pp.tile([PT, NE, L], FP8)  # xn^T [e, l]
            # K^T zero-padded per head parity: full-K=128 QK matmuls with
            # the other head's rows zeroed.
            kTzA = pp.tile([PT, NE, L], BF16)
            kTzB = pp.tile([PT, NE, L], BF16)
            nc.sync.dma_start(
                kTzA[64:128, :, :].rearrange("p a b -> p (a b)"),
                zz_d[0:1, :].partition_broadcast(64))
            nc.sync.dma_start(
                kTzB[0:64, :, :].rearrange("p a b -> p (a b)"),
                zz_d[0:1, :].partition_broadcast(64))
            # V | (ones * 2^-5) col per head; [p, cp, cparity, h, 65] fp8
            v3 = pp.tile([PT, CP, 2, H, 65], FP8)
            qT = pp.tile([PT, NE, QL], BF16)    # Q^T (scaled) [e_q, q]
            oT = pp.tile([PT, NE, QL], FP8)     # attnout^T * 32
            nc.gpsimd.memset(v3[:, :, :, :, 64:65], 2.0**-5)
            pbr = {}
            for name in pb_d:
                pbr[name] = cp.tile([1, E], BF16)
                nc.sync.dma_start(pbr[name], pb_d[name][:, :])

            # ================= Phase 1: LayerNorm + transpose ============
            with (
                tc.tile_pool(name="ln", bufs=3) as lp,
                tc.tile_pool(name="pst", bufs=4, space="PSUM") as ptp,
            ):
                for lt in range(NL):
                    xr = x_sb[:, lt, :].rearrange("p (s d) -> p s d", s=2)
                    stats = lp.tile([PT, 2, 6], F32, tag="stats")
                    for sg in range(2):
                        nc.vector.bn_stats(stats[:, sg, :], xr[:, sg, :])
                    mv = lp.tile([PT, 2], F32, tag="mv")
                    nc.vector.bn_aggr(mv, stats)
                    sd = lp.tile([PT, 1], F32, tag="sd")
                    nc.scalar.activation(sd, mv[:, 1:2],
                                         mybir.ActivationFunctionType.Sqrt,
                                         bias=eps_t)
                    rs = lp.tile([PT, 1], F32, tag="rs")
                    nc.vector.reciprocal(rs, sd)
                    xnb = lp.tile([PT, L], BF16, tag="xnb")
                    nc.gpsimd.tensor_scalar(
                        out=xnb, in0=x_sb[:, lt, :], scalar1=mv[:, 0:1],
                        scalar2=rs, op0=mybir.AluOpType.subtract,
                        op1=mybir.AluOpType.mult)
                    for g in range(2):
                        psT = ptp.tile([PT, QL], BF16, tag="psT")
                        for j in range(4):
                            et = g * 4 + j
                            nc.tensor.transpose(
                                psT[:, j * PT:(j + 1) * PT],
                                xnb[:, et * PT:(et + 1) * PT], ident)
                        dst = xnT[:, g * 4:(g + 1) * 4,
                                  lt * PT:(lt + 1) * PT]
                        src = psT.rearrange("p (j l) -> p j l", j=4)
                        if g == 0:
                            nc.vector.tensor_copy(dst, src)
                        else:
                            nc.scalar.copy(dst, src)

            # ======== Phase 2+3: projections pipelined w/ attention ======
            with (
                tc.tile_pool(name="work", bufs=2, space="PSUM") as wkp,
                tc.tile_pool(name="av", bufs=2, space="PSUM") as avp,
                tc.tile_pool(name="bias", bufs=4) as bp,
                tc.tile_pool(name="attn", bufs=4) as ap,
                tc.tile_pool(name="rec", bufs=2) as rcp,
                tc.tile_pool(name="oo", bufs=3) as oop,
                tc.tile_pool(name="yo", bufs=2) as yop,
                tc.tile_pool(name="recd", bufs=2, space="DRAM") as rdp,
            ):
                def k_proj(ot):
                    """K^T chunk ot: both l-halves into one [128,2,512]."""
                    osl = slice(ot * PT, (ot + 1) * PT)
                    ps = wkp.tile([PT, 2, QL], F32, tag="w")
                    for nh in range(2):
                        for kc in range(4):
                            nc.tensor.matmul(
                                ps[:, nh, :],
                                wk_sb[:, 2 * kc:2 * kc + 2, osl],
                                xnT[:, 2 * kc:2 * kc + 2,
                                    nh * QL:(nh + 1) * QL],
                                start=(kc == 0),
                                stop=(kc == 3 and "k" not in pbr),
                                perf_mode=DR)
                        if "k" in pbr:
                            nc.tensor.matmul(ps[:, nh, :], pbr["k"][:, osl],
                                             ones_row[:, 0:QL],
                                             start=False, stop=True)
                    psf = ps.rearrange("p a b -> p (a b)")
                    nc.vector.tensor_scalar_mul(
                        kTzA[0:64, ot, :], psf[0:64, :], ik)
                    nc.scalar.mul(kTzB[64:128, ot, :], psf[64:128, :], ik)

                def q_proj(ot):
                    osl = slice(ot * PT, (ot + 1) * PT)
                    psq = wkp.tile([PT, 2, QL], F32, tag="w")
                    for kc in range(4):
                        nc.tensor.matmul(
                            psq[:, 0, :], wq_sb[:, 2 * kc:2 * kc + 2, osl],
                            xnT[:, 2 * kc:2 * kc + 2, 0:QL],
                            start=(kc == 0),
                            stop=(kc == 3 and "q" not in pbr),
                            perf_mode=DR)
                    if "q" in pbr:
                        nc.tensor.matmul(psq[:, 0, :], pbr["q"][:, osl],
                                         ones_row[:, 0:QL],
                                         start=False, stop=True)
                    nc.vector.tensor_scalar_mul(
                        qT[:, ot, :], psq[:, 0, :], iq)

                def v_proj(lt, vh):
                    """V rows l-chunk lt, heads vh*8..vh*8+8 -> v3 fp8."""
                    lsl = slice(lt * PT, (lt + 1) * PT)
                    vsl = slice(vh * QL, (vh + 1) * QL)
                    psv = wkp.tile([PT, 2, QL], F32, tag="w")
                    for kc in range(4):
                        nc.tensor.matmul(
                            psv[:, 0, :],
                            xnT[:, 2 * kc:2 * kc + 2, lsl],
                            wv_sb[:, 2 * kc:2 * kc + 2, vsl],
                            start=(kc == 0),
                            stop=(kc == 3 and "v" not in pbr),
                            perf_mode=DR)
                    if "v" in pbr:
                        nc.tensor.matmul(psv[:, 0, :], ones_row[:, 0:PT],
                                         pbr["v"][:, vsl],
                                         start=False, stop=True)
                    dst = v3[:, lt // 2, lt % 2, vh * 8:(vh + 1) * 8, 0:64]
                    src = psv[:, 0, :].rearrange("p (h d) -> p h d", h=8)
                    if lt % 2 == 0:
                        nc.vector.tensor_scalar_mul(dst, src, iv)
                    else:
                        nc.scalar.mul(dst, src, iv)

                def attention(t, fillers=()):
                    """Head pair t. fillers: list of closures, one popped
                    per c-iteration and emitted after the QK matmuls so the
                    PE has work while vector/scalar produce `at`."""
                    fillers = list(fillers)
                    hA = 2 * t
                    av2 = avp.tile([65, 2, QL], F32, tag="av")
                    at = None
                    for c in range(NL):
                        cpi, cpar = divmod(c, 2)
                        csl = slice(c * PT, (c + 1) * PT)
                        if cpar == 0:
                            bt = bp.tile([PT, 2, 2, QL], FP8, tag="bt")
                            nc.sync.dma_start(bt, bias_d[t, cpi])
                            at = ap.tile([PT, 2, 2, QL], FP8, tag="at")
                        ps = wkp.tile([PT, 2, QL], F32, tag="w")
                        nc.tensor.matmul(ps[:, 0, :], kTzA[:, t, csl],
                                         qT[:, t, :], start=True, stop=True)
                        nc.tensor.matmul(ps[:, 1, :], kTzB[:, t, csl],
                                         qT[:, t, :], start=True, stop=True)
                        if fillers:
                            fillers.pop(0)()
                        psw = ps.rearrange("p h q -> p (h q)")
                        nc.vector.tensor_add(
                            psw, psw,
                            bt[:, cpar, :, :].rearrange("p h q -> p (h q)"))
                        kmb = km_sb[:, c:c + 1] if use_mask else shift_t
                        if same_gate:
                            nc.scalar.activation(
                                at[:, cpar, :, :].rearrange(
                                    "p h q -> p (h q)"),
                                ps.rearrange("p h q -> p (h q)"),
                                mybir.ActivationFunctionType.Exp,
                                bias=kmb, scale=gates[hA])
                        else:
                            for hi in range(2):
                                nc.scalar.activation(
                                    at[:, cpar, hi, :], ps[:, hi, :],
                                    mybir.ActivationFunctionType.Exp,
                                    bias=kmb, scale=gates[hA + hi])
                        if cpar == 1:
                            for hi in range(2):
                                nc.tensor.matmul(
                                    av2[:, hi, :],
                                    v3[:, cpi, :, hA + hi, :],
                                    at[:, :, hi, :],
                                    start=(cpi == 0), stop=(cpi == CP - 1),
                                    perf_mode=DR)
                    # normalize: rowsum rows -> recip -> broadcast -> mul
                    rec = rcp.tile([65, 2, QL], F32, tag="rec")
                    nc.scalar.copy(rec[64:65, :, :], av2[64:65, :, :])
                    recd = rdp.tile([1, 2 * QL], F32, tag="recd")
                    nc.sync.dma_start(
                        recd, rec[64:65, :, :].rearrange("p a b -> p (a b)"))
                    rbs = oop.tile([64, 2 * QL], F32, tag="rbs")
                    nc.sync.dma_start(rbs,
                                      recd[0:1, :].partition_broadcast(64))
                    nc.vector.reciprocal_approx_fast(out=rbs, in_=rbs)
                    if use_mask:
                        nc.gpsimd.tensor_mul(rbs, rbs, mqb)
                    rbs2 = rbs.rearrange("p (a b) -> p a b", a=2)
                    nc.vector.tensor_mul(oT[0:64, t, :], av2[0:64, 0, :],
                                         rbs2[:, 0, :])
                    ot_odd = oop.tile([64, QL], FP8, tag="oo")
                    nc.vector.tensor_mul(ot_odd, av2[0:64, 1, :],
                                         rbs2[:, 1, :])
                    nc.sync.dma_start(oT[64:128, t, :], ot_odd)

                for ot in range(4):
                    k_proj(ot)
                    q_proj(ot)
                for lt in range(NL):
                    v_proj(lt, 0)
                # remaining projection work, fed into the attention loops
                # one unit per c-iteration to keep the PE busy while
                # vector/scalar produce `at`
                units = []
                for ot in range(4, NE):
                    units.append(lambda ot=ot: k_proj(ot))
                    units.append(lambda ot=ot: q_proj(ot))
                for lt in range(NL):
                    units.append(lambda lt=lt: v_proj(lt, 1))
                attention(0, units[0:8])    # K4 Q4 K5 Q5 K6 Q6 K7 Q7
                attention(1, units[8:16])   # V vh=1 lt 0..7
                for t in range(2, HP):
                    attention(t)

                # ====== Phase 4: out-proj in [q, e] + residual ===========
                # final[q,e] = io/32 * sum_i oT32[i,q] * woT_s[i,e] + x[q,e]
                rescale = io / 32.0
                for qb in range(4):
                    qsl = slice(qb * PT, (qb + 1) * PT)
                    psf = wkp.tile([PT, 2, QL], F32, tag="w")
                    for eh in range(2):
                        esl = slice(eh * QL, (eh + 1) * QL)
                        for j in range(4):
                            nc.tensor.matmul(
                                psf[:, eh, :], oT[:, 2 * j:2 * j + 2, qsl],
                                wo_sb[:, 2 * j:2 * j + 2, esl],
                                start=(j == 0),
                                stop=(j == 3 and "o" not in pbr),
                                perf_mode=DR)
                        if "o" in pbr:
                            nc.tensor.matmul(psf[:, eh, :],
                                             ones_row[0:1, 0:PT],
                                             pbr["o"][:, esl],
                                             start=False, stop=True)
                    y_sb = yop.tile([PT, E], F32, tag="y")
                    nc.vector.scalar_tensor_tensor(
                        out=y_sb, in0=psf.rearrange("p a b -> p (a b)"),
                        scalar=rescale, in1=x_sb[:, qb, :],
                        op0=mybir.AluOpType.mult, op1=mybir.AluOpType.add)
                    nc.sync.dma_start(y_d[qsl, :], y_sb)
    return nc


def _fp8_scale(w):
    """Power-of-2 scale s so absmax(w*s) ~ 100 (fp8e4 max 240)."""
    am = float(np.max(np.abs(w)))
    if am == 0.0 or not np.isfinite(am):
        return 1.0
    return 2.0 ** math.floor(math.log2(100.0 / am))


def _prep_inputs(x, bias, mask, wq, bq, wk, bk, wv, bv, wo, bo, gate,
                 ln_g, ln_b):
    """Host-side folding + per-core sharding. Returns (in_maps, meta)."""
    gate = np.asarray(gate, np.float32)
    ln_g = np.asarray(ln_g, np.float32)
    ln_b = np.asarray(ln_b, np.float32)
    grep = np.repeat(gate, D)  # [E]
    safe_gate = bool(np.all(np.abs(gate) > 1e-6))
    if safe_gate:
        qscale = (SCALE / grep).astype(np.float32)
        exp_scales = [float(g) for g in gate]
    else:
        # fold gate into bias on host instead (gate ~ 0 edge case)
        qscale = np.full(E, SCALE, np.float32)
        exp_scales = [1.0] * H

    wqt = np.asarray(wq).T * ln_g[:, None] * qscale[None, :]
    wkt = np.asarray(wk).T * ln_g[:, None]
    wvt = np.asarray(wv).T * ln_g[:, None]
    wot = np.asarray(wo).T
    # fp8 scaling: weights scaled into fp8 range; inverse folded into the
    # PSUM->SBUF copies (q,k,v) or the final residual add (o). The out-proj
    # additionally sees oT at 32x natural (ones-col = 2^-5 rowsum trick).
    scales = {"q": _fp8_scale(wqt), "k": _fp8_scale(wkt),
              "v": _fp8_scale(wvt), "o": _fp8_scale(wot)}
    inv_scales = {n: 1.0 / s for n, s in scales.items()}
    wqt = (wqt * scales["q"]).astype(FP8_NP)
    wkt = (wkt * scales["k"]).astype(FP8_NP)
    wvt = (wvt * scales["v"]).astype(FP8_NP)
    wot = (wot * scales["o"]).astype(FP8_NP)
    bqe = ((np.asarray(wq) @ ln_b + np.asarray(bq)) * qscale
           * scales["q"]).astype(np.float32)
    bke = ((np.asarray(wk) @ ln_b + np.asarray(bk))
           * scales["k"]).astype(np.float32)
    bve = ((np.asarray(wv) @ ln_b + np.asarray(bv))
           * scales["v"]).astype(np.float32)
    boe = (np.asarray(bo, np.float32) * scales["o"] * 32.0)
    use_pbias = tuple(bool(np.any(b)) for b in (bqe, bke, bve, boe))

    mask = np.asarray(mask, np.int32)
    use_mask = not bool(np.all(mask == 1))

    def wfmt(w):  # [E_in, E_out] -> [128, 8, E]
        return np.ascontiguousarray(
            w.reshape(NE, PT, E).transpose(1, 0, 2))

    shared = {"wqt": wfmt(wqt), "wkt": wfmt(wkt), "wvt": wfmt(wvt),
              "wot": wfmt(wot),
              "zz": np.zeros((1, NE * L), BF_NP)}
    for name, use, b in zip("qkvo", use_pbias, (bqe, bke, bve, boe)):
        if use:
            shared[f"b{name}e"] = b.reshape(1, E).astype(BF_NP)

    x = np.asarray(x, np.float32)
    bias = np.asarray(bias, np.float32)
    in_maps = []
    for c in range(NCORES):
        b_idx, qh = divmod(c, 2)
        q0 = qh * QL
        xr = np.roll(x[b_idx], -q0, axis=0)  # query block first
        m = {}
        m.update(shared)
        m["xc"] = np.ascontiguousarray(
            xr.reshape(NL, PT, L).transpose(1, 0, 2))
        bs = bias[b_idx][:, q0:q0 + QL, :]  # [H, QL, L]
        bs = np.roll(bs, -q0, axis=2)       # roll key axis
        if not safe_gate:
            bs = bs * gate[:, None, None]
        # [H,(t,hp), q, k=(cp,cpar,p)] -> [t, cp, p, cpar, hp, q]
        b6 = bs.reshape(HP, 2, QL, CP, 2, PT).transpose(0, 3, 5, 4, 1, 2)
        m["biasc"] = np.ascontiguousarray(b6).astype(FP8_NP)
        if use_mask:
            mr = np.roll(mask[b_idx], -q0)
            kmf = (-10000.0 * (1.0 - mr.astype(np.float32))) - SHIFT
            m["kmc"] = np.ascontiguousarray(
                kmf.reshape(NL, PT).T).astype(np.float32)
            mq = mr[:QL].astype(np.float32)
            m["mqc"] = np.tile(mq, 2).reshape(1, 2 * QL)
        in_maps.append(m)
    return in_maps, (exp_scales, inv_scales, use_pbias, use_mask)


def kernel(**inputs):
    global LAST_RESULT
    in_maps, (exp_scales, inv_scales, use_pbias, use_mask) = \
        _prep_inputs(**inputs)
    nc = _build_nc(exp_scales, inv_scales, use_pbias, use_mask)
    if not nc.is_finalized():
        nc.finalize()
    res = run_bass_kernel_spmd(nc, in_maps, core_ids=list(range(NCORES)))
    LAST_RESULT = res
    out = np.empty((B, L, E), np.float32)
    for c in range(NCORES):
        b_idx, qh = divmod(c, 2)
        out[b_idx, qh * QL:(qh + 1) * QL, :] = res.results[c]["yc"]
    return out


# revision 15
# speedup vs baseline: 1.3561x; 1.3561x over previous
"""BiasedMultiHeadAttention Trainium2 kernel (fp8 DoubleRow pipeline).

Sharding: 8 cores = (batch b, query-half qh). Each core computes the full
pipeline for its 512 query rows of batch b (K/V projections for the batch
are duplicated across the 2 cores sharing it). No collectives.

Device layout trick: per-core x rows are host-rolled so the core's query
block is always rows 0..511 -> one SPMD program for all 8 cores; bias/mask
are rolled consistently (softmax sum order irrelevant).

Perf structure:
  - All projections + AV run as fp8e4 DoubleRow matmuls (2x PE rate).
    Weights are host-scaled by a power of two into fp8 range; the inverse
    scale is folded into the PSUM->SBUF copy (tensor_scalar mul) or the
    final residual add, so numerics stay at natural scale in bf16/fp32.
  - QK stays bf16 (output-bound: free-dim cycles dominate either way).
  - Attention bias-add happens in-place in PSUM, split across Vector
    (head A) and GpSimd (head B); Scalar engine does only the exp.
  - exp computed with a constant -SHIFT bias so fp8 'at' can't overflow;
    the shift cancels exactly in the rowsum normalization.
  - Emission interleaves attention head-pairs 0..2 with the remaining
    projection matmuls so the PE never drains.

Math folding (host, exact):
  xn_aff = ln(x)*g + b folded into weights:  w_eff[i,o] = w[o,i]*ln_g[i]
  b_eff[o] = (w @ ln_b + b)[o];  Q additionally scaled by SCALE/gate_h and
  exp computed as exp(gate_h * s + key_mask - SHIFT) via ACT operands.
"""

import math

import numpy as np
import ml_dtypes

import concourse.bass as bass
import concourse.tile as tile
import concourse.mybir as mybir
from concourse import bacc
from concourse.bass_utils import run_bass_kernel_spmd
from concourse.masks import make_identity

B, L, E, H = 4, 1024, 1024, 16
D = E // H
SCALE = D**-0.5
EPS = 1e-5
NCORES = 8
QL = 512  # query rows per core
PT = 128  # partitions
NL = L // PT  # 8 l-chunks
NE = E // PT  # 8 e-chunks
HP = H // 2  # 8 head pairs
CP = NL // 2  # 4 key-chunk pairs
SHIFT = 1.5  # exp(x - SHIFT); cancels in normalization

F32 = mybir.dt.float32
BF16 = mybir.dt.bfloat16
FP8 = mybir.dt.float8e4
I32 = mybir.dt.int32
BF_NP = ml_dtypes.bfloat16
FP8_NP = ml_dtypes.float8_e4m3
DR = mybir.MatmulPerfMode.DoubleRow

LAST_RESULT = None  # BassKernelResults of the most recent run (for test.py)


def _build_nc(gates, inv_scales, use_pbias, use_mask):
    """Build the single-core Bass program (same NEFF for all 8 cores).

    gates: 16 python floats (exp scale immediates)
    inv_scales: dict name -> float, inverse of the host fp8 weight scaling
    use_pbias: 4 bools - include projection-bias rank-1 matmuls for q,k,v,o
    use_mask: include key/query mask handling
    """
    nc = bacc.Bacc("TRN2", target_bir_lowering=False, debug=False)

    x_d = nc.dram_tensor("xc", [PT, NL, L], F32, kind="ExternalInput")
    bias_d = nc.dram_tensor("biasc", [HP, CP, PT, 2, 2, QL], FP8,
                            kind="ExternalInput")
    wq_d = nc.dram_tensor("wqt", [PT, NE, E], FP8, kind="ExternalInput")
    wk_d = nc.dram_tensor("wkt", [PT, NE, E], FP8, kind="ExternalInput")
    wv_d = nc.dram_tensor("wvt", [PT, NE, E], FP8, kind="ExternalInput")
    wo_d = nc.dram_tensor("wot", [PT, NE, E], FP8, kind="ExternalInput")
    zz_d = nc.dram_tensor("zz", [1, NE * L], BF16, kind="ExternalInput")
    pb_d = {}
    for name, use in zip("qkvo", use_pbias):
        if use:
            pb_d[name] = nc.dram_tensor(f"b{name}e", [1, E], BF16,
                                        kind="ExternalInput")
    if use_mask:
        km_d = nc.dram_tensor("kmc", [PT, NL], F32, kind="ExternalInput")
        mq_d = nc.dram_tensor("mqc", [1, 2 * QL], F32, kind="ExternalInput")
    y_d = nc.dram_tensor("yc", [QL, E], F32, kind="ExternalOutput")

    iq, ik, iv, io = (inv_scales[n] for n in "qkvo")
    same_gate = len(set(gates)) == 1

    with tile.TileContext(nc) as tc:
        with (
            tc.tile_pool(name="persist", bufs=1) as pp,
            tc.tile_pool(name="consts", bufs=1) as cp,
        ):
            # ---- constants ----
            ident = cp.tile([PT, PT], BF16)
            make_identity(nc, ident)
            ones_row = cp.tile([1, L], BF16)
            nc.vector.memset(ones_row, 1.0)
            eps_t = cp.tile([PT, 1], F32)
            nc.vector.memset(eps_t, EPS)
            shift_t = cp.tile([PT, 1], F32)
            nc.vector.memset(shift_t, -SHIFT)
            identF8 = cp.tile([PT, PT], FP8)
            make_identity(nc, identF8)
            ones64 = cp.tile([1, 64], BF16)
            nc.vector.memset(ones64, 1.0)
            if use_mask:
                km_sb = cp.tile([PT, NL], F32)
                nc.sync.dma_start(km_sb, km_d[:, :])
                mqb = cp.tile([64, 2 * QL], F32)
                nc.gpsimd.dma_start(mqb,
                                    mq_d[0:1, :].partition_broadcast(64))

            # ---- resident tensors ----
            x_sb = pp.tile([PT, NL, L], F32)
            for lt in range(NL):
                nc.sync.dma_start(x_sb[:, lt, :], x_d[:, lt, :])
            wq_sb = pp.tile([PT, NE, E], FP8)
            nc.sync.dma_start(wq_sb, wq_d[:, :, :])
            wk_sb = pp.tile([PT, NE, E], FP8)
            nc.sync.dma_start(wk_sb, wk_d[:, :, :])
            wv_sb = pp.tile([PT, NE, E], FP8)
            nc.sync.dma_start(wv_sb, wv_d[:, :, :])
            wo_sb = pp.tile([PT, NE, E], FP8)
            nc.sync.dma_start(wo_sb, wo_d[:, :, :])
            xnT = pp.tile([PT, NE, L], FP8)  # xn^T [e, l]
            # K^T zero-padded per head parity: full-K=128 QK matmuls with
            # the other head's rows zeroed.
            kTzA = pp.tile([PT, NE, L], BF16)
            kTzB = pp.tile([PT, NE, L], BF16)
            nc.sync.dma_start(
                kTzA[64:128, :, :].rearrange("p a b -> p (a b)"),
                zz_d[0:1, :].partition_broadcast(64))
            nc.sync.dma_start(
                kTzB[0:64, :, :].rearrange("p a b -> p (a b)"),
                zz_d[0:1, :].partition_broadcast(64))
            # V | (ones * 2^-5) col per head; [p, cp, cparity, h, 65] fp8
            v3 = pp.tile([PT, CP, 2, H, 65], FP8)
            qT = pp.tile([PT, NE, QL], BF16)    # Q^T (scaled) [e_q, q]
            oT = pp.tile([PT, NE, QL], FP8)     # attnout^T * 32
            nc.gpsimd.memset(v3[:, :, :, :, 64:65], 2.0**-5)
            pbr = {}
            for name in pb_d:
                pbr[name] = cp.tile([1, E], BF16)
                nc.sync.dma_start(pbr[name], pb_d[name][:, :])

            # ================= Phase 1: LayerNorm + transpose ============
            with (
                tc.tile_pool(name="ln", bufs=3) as lp,
                tc.tile_pool(name="pst", bufs=4, space="PSUM") as ptp,
            ):
                for lt in range(NL):
                    xr = x_sb[:, lt, :].rearrange("p (s d) -> p s d", s=2)
                    stats = lp.tile([PT, 2, 6], F32, tag="stats")
                    for sg in range(2):
                        nc.vector.bn_stats(stats[:, sg, :], xr[:, sg, :])
                    mv = lp.tile([PT, 2], F32, tag="mv")
                    nc.vector.bn_aggr(mv, stats)
                    sd = lp.tile([PT, 1], F32, tag="sd")
                    nc.scalar.activation(sd, mv[:, 1:2],
                                         mybir.ActivationFunctionType.Sqrt,
                                         bias=eps_t)
                    rs = lp.tile([PT, 1], F32, tag="rs")
                    nc.vector.reciprocal(rs, sd)
                    xnb = lp.tile([PT, L], BF16, tag="xnb")
                    nc.vector.tensor_scalar(
                        out=xnb, in0=x_sb[:, lt, :], scalar1=mv[:, 0:1],
                        scalar2=rs, op0=mybir.AluOpType.subtract,
                        op1=mybir.AluOpType.mult)
                    for g in range(2):
                        psT = ptp.tile([PT, QL], BF16, tag="psT")
                        for j in range(4):
                            et = g * 4 + j
                            nc.tensor.transpose(
                                psT[:, j * PT:(j + 1) * PT],
                                xnb[:, et * PT:(et + 1) * PT], ident)
                        dst = xnT[:, g * 4:(g + 1) * 4,
                                  lt * PT:(lt + 1) * PT]
                        src = psT.rearrange("p (j l) -> p j l", j=4)
                        if g == 0:
                            nc.vector.tensor_copy(dst, src)
                        else:
                            nc.scalar.copy(dst, src)

            # ======== Phase 2+3: projections pipelined w/ attention ======
            with (
                tc.tile_pool(name="work", bufs=2, space="PSUM") as wkp,
                tc.tile_pool(name="av", bufs=2, space="PSUM") as avp,
                tc.tile_pool(name="bias", bufs=4) as bp,
                tc.tile_pool(name="attn", bufs=4) as ap,
                tc.tile_pool(name="rec", bufs=2) as rcp,
                tc.tile_pool(name="oo", bufs=3) as oop,
                tc.tile_pool(name="yo", bufs=2) as yop,
            ):
                def k_proj(ot):
                    """K^T chunk ot: both l-halves into one [128,2,512]."""
                    osl = slice(ot * PT, (ot + 1) * PT)
                    ps = wkp.tile([PT, 2, QL], F32, tag="w")
                    for nh in range(2):
                        for kc in range(4):
                            nc.tensor.matmul(
                                ps[:, nh, :],
                                wk_sb[:, 2 * kc:2 * kc + 2, osl],
                                xnT[:, 2 * kc:2 * kc + 2,
                                    nh * QL:(nh + 1) * QL],
                                start=(kc == 0),
                                stop=(kc == 3 and "k" not in pbr),
                                perf_mode=DR)
                        if "k" in pbr:
                            nc.tensor.matmul(ps[:, nh, :], pbr["k"][:, osl],
                                             ones_row[:, 0:QL],
                                             start=False, stop=True)
                    psf = ps.rearrange("p a b -> p (a b)")
                    nc.vector.tensor_scalar_mul(
                        kTzA[0:64, ot, :], psf[0:64, :], ik)
                    nc.scalar.mul(kTzB[64:128, ot, :], psf[64:128, :], ik)

                def q_proj(ot):
                    osl = slice(ot * PT, (ot + 1) * PT)
                    psq = wkp.tile([PT, 2, QL], F32, tag="w")
                    for kc in range(4):
                        nc.tensor.matmul(
                            psq[:, 0, :], wq_sb[:, 2 * kc:2 * kc + 2, osl],
                            xnT[:, 2 * kc:2 * kc + 2, 0:QL],
                            start=(kc == 0),
                            stop=(kc == 3 and "q" not in pbr),
                            perf_mode=DR)
                    if "q" in pbr:
                        nc.tensor.matmul(psq[:, 0, :], pbr["q"][:, osl],
                                         ones_row[:, 0:QL],
                                         start=False, stop=True)
                    nc.vector.tensor_scalar_mul(
                        qT[:, ot, :], psq[:, 0, :], iq)

                def v_proj(lt, vh):
                    """V rows l-chunk lt, heads vh*8..vh*8+8 -> v3 fp8."""
                    lsl = slice(lt * PT, (lt + 1) * PT)
                    vsl = slice(vh * QL, (vh + 1) * QL)
                    psv = wkp.tile([PT, 2, QL], F32, tag="w")
                    for kc in range(4):
                        nc.tensor.matmul(
                            psv[:, 0, :],
                            xnT[:, 2 * kc:2 * kc + 2, lsl],
                            wv_sb[:, 2 * kc:2 * kc + 2, vsl],
                            start=(kc == 0),
                            stop=(kc == 3 and "v" not in pbr),
                            perf_mode=DR)
                    if "v" in pbr:
                        nc.tensor.matmul(psv[:, 0, :], ones_row[:, 0:PT],
                                         pbr["v"][:, vsl],
                                         start=False, stop=True)
                    dst = v3[:, lt // 2, lt % 2, vh * 8:(vh + 1) * 8, 0:64]
                    src = psv[:, 0, :].rearrange("p (h d) -> p h d", h=8)
                    if lt % 2 == 0:
                        nc.vector.tensor_scalar_mul(dst, src, iv)
                    else:
                        nc.scalar.mul(dst, src, iv)

                def attention(t, fillers=()):
                    """Head pair t. fillers: list of closures, one popped
                    per c-iteration and emitted after the QK matmuls so the
                    PE has work while vector/scalar produce `at`."""
                    fillers = list(fillers)
                    hA = 2 * t
                    av2 = avp.tile([65, 2, QL], F32, tag="av")
                    at = None
                    for c in range(NL):
                        cpi, cpar = divmod(c, 2)
                        csl = slice(c * PT, (c + 1) * PT)
                        if cpar == 0:
                            bt = bp.tile([PT, 2, 2, QL], FP8, tag="bt")
                            nc.sync.dma_start(bt, bias_d[t, cpi])
                            at = ap.tile([PT, 2, 2, QL], FP8, tag="at")
                        ps = wkp.tile([PT, 2, QL], F32, tag="w")
                        nc.tensor.matmul(ps[:, 0, :], kTzA[:, t, csl],
                                         qT[:, t, :], start=True, stop=False)
                        nc.tensor.matmul(ps[:, 0, :], identF8,
                                         bt[:, cpar, 0, :],
                                         start=False, stop=True)
                        nc.tensor.matmul(ps[:, 1, :], kTzB[:, t, csl],
                                         qT[:, t, :], start=True, stop=False)
                        nc.tensor.matmul(ps[:, 1, :], identF8,
                                         bt[:, cpar, 1, :],
                                         start=False, stop=True)
                        if fillers:
                            fillers.pop(0)()
                        kmb = km_sb[:, c:c + 1] if use_mask else shift_t
                        if same_gate:
                            nc.scalar.activation(
                                at[:, cpar, :, :].rearrange(
                                    "p h q -> p (h q)"),
                                ps.rearrange("p h q -> p (h q)"),
                                mybir.ActivationFunctionType.Exp,
                                bias=kmb, scale=gates[hA])
                        else:
                            for hi in range(2):
                                nc.scalar.activation(
                                    at[:, cpar, hi, :], ps[:, hi, :],
                                    mybir.ActivationFunctionType.Exp,
                                    bias=kmb, scale=gates[hA + hi])
                        if cpar == 1:
                            for hi in range(2):
                                nc.tensor.matmul(
                                    av2[:, hi, :],
                                    v3[:, cpi, :, hA + hi, :],
                                    at[:, :, hi, :],
                                    start=(cpi == 0), stop=(cpi == CP - 1),
                                    perf_mode=DR)
                    # normalize: rowsum row -> PE outer-product broadcast
                    # -> recip -> mul
                    rec = rcp.tile([1, 2, QL], BF16, tag="rec")
                    nc.vector.tensor_copy(rec, av2[64:65, :, :])
                    rbsp = wkp.tile([PT, 2, QL], F32, tag="w")
                    for hi in range(2):
                        nc.tensor.matmul(rbsp[0:64, hi, :], ones64,
                                         rec[:, hi, :],
                                         start=True, stop=True)
                    rbs = oop.tile([64, 2 * QL], F32, tag="rbs")
                    nc.vector.reciprocal(
                        rbs, rbsp[0:64, :, :].rearrange("p a b -> p (a b)"))
                    if use_mask:
                        nc.gpsimd.tensor_mul(rbs, rbs, mqb)
                    rbs2 = rbs.rearrange("p (a b) -> p a b", a=2)
                    nc.vector.tensor_mul(oT[0:64, t, :], av2[0:64, 0, :],
                                         rbs2[:, 0, :])
                    ot_odd = oop.tile([64, QL], FP8, tag="oo")
                    nc.vector.tensor_mul(ot_odd, av2[0:64, 1, :],
                                         rbs2[:, 1, :])
                    nc.sync.dma_start(oT[64:128, t, :], ot_odd)

                for ot in range(4):
                    k_proj(ot)
                    q_proj(ot)
                for lt in range(NL):
                    v_proj(lt, 0)
                # remaining projection work, fed into the attention loops
                # one unit per c-iteration to keep the PE busy while
                # vector/scalar produce `at`
                units = []
                for ot in range(4, NE):
                    units.append(lambda ot=ot: k_proj(ot))
                    units.append(lambda ot=ot: q_proj(ot))
                for lt in range(NL):
                    units.append(lambda lt=lt: v_proj(lt, 1))
                attention(0, units[0:8])    # K4 Q4 K5 Q5 K6 Q6 K7 Q7
                attention(1, units[8:16])   # V vh=1 lt 0..7
                for t in range(2, HP):
                    attention(t)

                # ====== Phase 4: out-proj in [q, e] + residual ===========
                # final[q,e] = io/32 * sum_i oT32[i,q] * woT_s[i,e] + x[q,e]
                rescale = io / 32.0
                for qb in range(4):
                    qsl = slice(qb * PT, (qb + 1) * PT)
                    psf = wkp.tile([PT, 2, QL], F32, tag="w")
                    for eh in range(2):
                        esl = slice(eh * QL, (eh + 1) * QL)
                        for j in range(4):
                            nc.tensor.matmul(
                                psf[:, eh, :], oT[:, 2 * j:2 * j + 2, qsl],
                                wo_sb[:, 2 * j:2 * j + 2, esl],
                                start=(j == 0),
                                stop=(j == 3 and "o" not in pbr),
                                perf_mode=DR)
                        if "o" in pbr:
                            nc.tensor.matmul(psf[:, eh, :],
                                             ones_row[0:1, 0:PT],
                                             pbr["o"][:, esl],
                                             start=False, stop=True)
                    y_sb = yop.tile([PT, E], F32, tag="y")
                    nc.vector.scalar_tensor_tensor(
                        out=y_sb, in0=psf.rearrange("p a b -> p (a b)"),
                        scalar=rescale, in1=x_sb[:, qb, :],
                        op0=mybir.AluOpType.mult, op1=mybir.AluOpType.add)
                    nc.sync.dma_start(y_d[qsl, :], y_sb)
    return nc


def _fp8_scale(w):
    """Power-of-2 scale s so absmax(w*s) ~ 100 (fp8e4 max 240)."""
    am = float(np.max(np.abs(w)))
    if am == 0.0 or not np.isfinite(am):
        return 1.0
    return 2.0 ** math.floor(math.log2(100.0 / am))


def _prep_inputs(x, bias, mask, wq, bq, wk, bk, wv, bv, wo, bo, gate,
                 ln_g, ln_b):
    """Host-side folding + per-core sharding. Returns (in_maps, meta)."""
    gate = np.asarray(gate, np.float32)
    ln_g = np.asarray(ln_g, np.float32)
    ln_b = np.asarray(ln_b, np.float32)
    grep = np.repeat(gate, D)  # [E]
    safe_gate = bool(np.all(np.abs(gate) > 1e-6))
    if safe_gate:
        qscale = (SCALE / grep).astype(np.float32)
        exp_scales = [float(g) for g in gate]
    else:
        # fold gate into bias on host instead (gate ~ 0 edge case)
        qscale = np.full(E, SCALE, np.float32)
        exp_scales = [1.0] * H

    wqt = np.asarray(wq).T * ln_g[:, None] * qscale[None, :]
    wkt = np.asarray(wk).T * ln_g[:, None]
    wvt = np.asarray(wv).T * ln_g[:, None]
    wot = np.asarray(wo).T
    # fp8 scaling: weights scaled into fp8 range; inverse folded into the
    # PSUM->SBUF copies (q,k,v) or the final residual add (o). The out-proj
    # additionally sees oT at 32x natural (ones-col = 2^-5 rowsum trick).
    scales = {"q": _fp8_scale(wqt), "k": _fp8_scale(wkt),
              "v": _fp8_scale(wvt), "o": _fp8_scale(wot)}
    inv_scales = {n: 1.0 / s for n, s in scales.items()}
    wqt = (wqt * scales["q"]).astype(FP8_NP)
    wkt = (wkt * scales["k"]).astype(FP8_NP)
    wvt = (wvt * scales["v"]).astype(FP8_NP)
    wot = (wot * scales["o"]).astype(FP8_NP)
    bqe = ((np.asarray(wq) @ ln_b + np.asarray(bq)) * qscale
           * scales["q"]).astype(np.float32)
    bke = ((np.asarray(wk) @ ln_b + np.asarray(bk))
           * scales["k"]).astype(np.float32)
    bve = ((np.asarray(wv) @ ln_b + np.asarray(bv))
           * scales["v"]).astype(np.float32)
    boe = (np.asarray(bo, np.float32) * scales["o"] * 32.0)
    use_pbias = tuple(bool(np.any(b)) for b in (bqe, bke, bve, boe))

    mask = np.asarray(mask, np.int32)
    use_mask = not bool(np.all(mask == 1))

    def wfmt(w):  # [E_in, E_out] -> [128, 8, E]
        return np.ascontiguousarray(
            w.reshape(NE, PT, E).transpose(1, 0, 2))

    shared = {"wqt": wfmt(wqt), "wkt": wfmt(wkt), "wvt": wfmt(wvt),
              "wot": wfmt(wot),
              "zz": np.zeros((1, NE * L), BF_NP)}
    for name, use, b in zip("qkvo", use_pbias, (bqe, bke, bve, boe)):
        if use:
            shared[f"b{name}e"] = b.reshape(1, E).astype(BF_NP)

    x = np.asarray(x, np.float32)
    bias = np.asarray(bias, np.float32)
    in_maps = []
    for c in range(NCORES):
        b_idx, qh = divmod(c, 2)
        q0 = qh * QL
        xr = np.roll(x[b_idx], -q0, axis=0)  # query block first
        m = {}
        m.update(shared)
        m["xc"] = np.ascontiguousarray(
            xr.reshape(NL, PT, L).transpose(1, 0, 2))
        bs = bias[b_idx][:, q0:q0 + QL, :]  # [H, QL, L]
        bs = np.roll(bs, -q0, axis=2)       # roll key axis
        if not safe_gate:
            bs = bs * gate[:, None, None]
        # [H,(t,hp), q, k=(cp,cpar,p)] -> [t, cp, p, cpar, hp, q]
        b6 = bs.reshape(HP, 2, QL, CP, 2, PT).transpose(0, 3, 5, 4, 1, 2)
        m["biasc"] = np.ascontiguousarray(b6).astype(FP8_NP)
        if use_mask:
            mr = np.roll(mask[b_idx], -q0)
            kmf = (-10000.0 * (1.0 - mr.astype(np.float32))) - SHIFT
            m["kmc"] = np.ascontiguousarray(
                kmf.reshape(NL, PT).T).astype(np.float32)
            mq = mr[:QL].astype(np.float32)
            m["mqc"] = np.tile(mq, 2).reshape(1, 2 * QL)
        in_maps.append(m)
    return in_maps, (exp_scales, inv_scales, use_pbias, use_mask)


def kernel(**inputs):
    global LAST_RESULT
    in_maps, (exp_scales, inv_scales, use_pbias, use_mask) = \
        _prep_inputs(**inputs)
    nc = _build_nc(exp_scales, inv_scales, use_pbias, use_mask)
    if not nc.is_finalized():
        nc.finalize()
    res = run_bass_kernel_spmd(nc, in_maps, core_ids=list(range(NCORES)))
    LAST_RESULT = res
    out = np.empty((B, L, E), np.float32)
    for c in range(NCORES):
        b_idx, qh = divmod(c, 2)
        out[b_idx, qh * QL:(qh + 1) * QL, :] = res.results[c]["yc"]
    return out


# revision 16
# speedup vs baseline: 1.6754x; 1.2354x over previous
"""BiasedMultiHeadAttention Trainium2 kernel (fp8 DoubleRow pipeline).

Sharding: 8 cores = (batch b, query-half qh). Each core computes the full
pipeline for its 512 query rows of batch b (K/V projections for the batch
are duplicated across the 2 cores sharing it). No collectives.

Device layout trick: per-core x rows are host-rolled so the core's query
block is always rows 0..511 -> one SPMD program for all 8 cores; bias/mask
are rolled consistently (softmax sum order irrelevant).

Perf structure:
  - All projections + AV run as fp8e4 DoubleRow matmuls (2x PE rate).
    Weights are host-scaled by a power of two into fp8 range; the inverse
    scale is folded into the PSUM->SBUF copy (tensor_scalar mul) or the
    final residual add, so numerics stay at natural scale in bf16/fp32.
  - QK stays bf16 (output-bound: free-dim cycles dominate either way).
  - Attention bias-add happens in-place in PSUM, split across Vector
    (head A) and GpSimd (head B); Scalar engine does only the exp.
  - exp computed with a constant -SHIFT bias so fp8 'at' can't overflow;
    the shift cancels exactly in the rowsum normalization.
  - Emission interleaves attention head-pairs 0..2 with the remaining
    projection matmuls so the PE never drains.

Math folding (host, exact):
  xn_aff = ln(x)*g + b folded into weights:  w_eff[i,o] = w[o,i]*ln_g[i]
  b_eff[o] = (w @ ln_b + b)[o];  Q additionally scaled by SCALE/gate_h and
  exp computed as exp(gate_h * s + key_mask - SHIFT) via ACT operands.
"""

import math

import numpy as np
import ml_dtypes

import concourse.bass as bass
import concourse.tile as tile
import concourse.mybir as mybir
from concourse import bacc
from concourse.bass_utils import run_bass_kernel_spmd
from concourse.masks import make_identity

B, L, E, H = 4, 1024, 1024, 16
D = E // H
SCALE = D**-0.5
EPS = 1e-5
NCORES = 8
QL = 512  # query rows per core
PT = 128  # partitions
NL = L // PT  # 8 l-chunks
NE = E // PT  # 8 e-chunks
HP = H // 2  # 8 head pairs
CP = NL // 2  # 4 key-chunk pairs
SHIFT = 1.5  # exp(x - SHIFT); cancels in normalization

F32 = mybir.dt.float32
BF16 = mybir.dt.bfloat16
FP8 = mybir.dt.float8e4
I32 = mybir.dt.int32
BF_NP = ml_dtypes.bfloat16
FP8_NP = ml_dtypes.float8_e4m3
DR = mybir.MatmulPerfMode.DoubleRow

LAST_RESULT = None  # BassKernelResults of the most recent run (for test.py)


def _build_nc(gates, inv_scales, use_pbias, use_mask):
    """Build the single-core Bass program (same NEFF for all 8 cores).

    gates: 16 python floats (exp scale immediates)
    inv_scales: dict name -> float, inverse of the host fp8 weight scaling
    use_pbias: 4 bools - include projection-bias rank-1 matmuls for q,k,v,o
    use_mask: include key/query mask handling
    """
    nc = bacc.Bacc("TRN2", target_bir_lowering=False, debug=False)

    x_d = nc.dram_tensor("xc", [PT, NL, L], F32, kind="ExternalInput")
    bias_d = nc.dram_tensor("biasc", [HP, CP, PT, 2, 2, QL], FP8,
                            kind="ExternalInput")
    wq_d = nc.dram_tensor("wqt", [PT, NE, E], FP8, kind="ExternalInput")
    wk_d = nc.dram_tensor("wkt", [PT, NE, E], FP8, kind="ExternalInput")
    wv_d = nc.dram_tensor("wvt", [PT, NE, E], FP8, kind="ExternalInput")
    wo_d = nc.dram_tensor("wot", [PT, NE, E], FP8, kind="ExternalInput")
    zz_d = nc.dram_tensor("zz", [1, NE * L], BF16, kind="ExternalInput")
    pb_d = {}
    for name, use in zip("qkvo", use_pbias):
        if use:
            pb_d[name] = nc.dram_tensor(f"b{name}e", [1, E], BF16,
                                        kind="ExternalInput")
    if use_mask:
        km_d = nc.dram_tensor("kmc", [PT, NL], F32, kind="ExternalInput")
        mq_d = nc.dram_tensor("mqc", [1, 2 * QL], F32, kind="ExternalInput")
    y_d = nc.dram_tensor("yc", [QL, E], F32, kind="ExternalOutput")

    iq, ik, iv, io = (inv_scales[n] for n in "qkvo")
    same_gate = len(set(gates)) == 1

    with tile.TileContext(nc) as tc:
        with (
            tc.tile_pool(name="persist", bufs=1) as pp,
            tc.tile_pool(name="consts", bufs=1) as cp,
        ):
            # ---- constants ----
            ident = cp.tile([PT, PT], BF16)
            make_identity(nc, ident)
            ones_row = cp.tile([1, L], BF16)
            nc.vector.memset(ones_row, 1.0)
            eps_t = cp.tile([PT, 1], F32)
            nc.vector.memset(eps_t, EPS)
            shift_t = cp.tile([PT, 1], F32)
            nc.vector.memset(shift_t, -SHIFT)
            if use_mask:
                km_sb = cp.tile([PT, NL], F32)
                nc.sync.dma_start(km_sb, km_d[:, :])
                mqb = cp.tile([64, 2 * QL], F32)
                nc.gpsimd.dma_start(mqb,
                                    mq_d[0:1, :].partition_broadcast(64))

            # ---- resident tensors ----
            x_sb = pp.tile([PT, NL, L], F32)
            for lt in range(NL):
                nc.sync.dma_start(x_sb[:, lt, :], x_d[:, lt, :])
            wq_sb = pp.tile([PT, NE, E], FP8)
            nc.sync.dma_start(wq_sb, wq_d[:, :, :])
            wk_sb = pp.tile([PT, NE, E], FP8)
            nc.sync.dma_start(wk_sb, wk_d[:, :, :])
            wv_sb = pp.tile([PT, NE, E], FP8)
            nc.sync.dma_start(wv_sb, wv_d[:, :, :])
            wo_sb = pp.tile([PT, NE, E], FP8)
            nc.sync.dma_start(wo_sb, wo_d[:, :, :])
            xnT = pp.tile([PT, NE, L], FP8)  # xn^T [e, l]
            # K^T zero-padded per head parity: full-K=128 QK matmuls with
            # the other head's rows zeroed.
            kTzA = pp.tile([PT, NE, L], BF16)
            kTzB = pp.tile([PT, NE, L], BF16)
            nc.sync.dma_start(
                kTzA[64:128, :, :].rearrange("p a b -> p (a b)"),
                zz_d[0:1, :].partition_broadcast(64))
            nc.sync.dma_start(
                kTzB[0:64, :, :].rearrange("p a b -> p (a b)"),
                zz_d[0:1, :].partition_broadcast(64))
            # V | (ones * 2^-5) col per head; [p, cp, cparity, h, 65] fp8
            v3 = pp.tile([PT, CP, 2, H, 65], FP8)
            qT = pp.tile([PT, NE, QL], BF16)    # Q^T (scaled) [e_q, q]
            oT = pp.tile([PT, NE, QL], FP8)     # attnout^T * 32
            nc.gpsimd.memset(v3[:, :, :, :, 64:65], 2.0**-5)
            pbr = {}
            for name in pb_d:
                pbr[name] = cp.tile([1, E], BF16)
                nc.sync.dma_start(pbr[name], pb_d[name][:, :])

            # ================= Phase 1: LayerNorm + transpose ============
            with (
                tc.tile_pool(name="ln", bufs=3) as lp,
                tc.tile_pool(name="pst", bufs=4, space="PSUM") as ptp,
            ):
                for lt in range(NL):
                    xr = x_sb[:, lt, :].rearrange("p (s d) -> p s d", s=2)
                    stats = lp.tile([PT, 2, 6], F32, tag="stats")
                    for sg in range(2):
                        nc.vector.bn_stats(stats[:, sg, :], xr[:, sg, :])
                    mv = lp.tile([PT, 2], F32, tag="mv")
                    nc.vector.bn_aggr(mv, stats)
                    sd = lp.tile([PT, 1], F32, tag="sd")
                    nc.scalar.activation(sd, mv[:, 1:2],
                                         mybir.ActivationFunctionType.Sqrt,
                                         bias=eps_t)
                    rs = lp.tile([PT, 1], F32, tag="rs")
                    nc.vector.reciprocal(rs, sd)
                    xnb = lp.tile([PT, L], BF16, tag="xnb")
                    nc.vector.tensor_scalar(
                        out=xnb, in0=x_sb[:, lt, :], scalar1=mv[:, 0:1],
                        scalar2=rs, op0=mybir.AluOpType.subtract,
                        op1=mybir.AluOpType.mult)
                    for g in range(2):
                        psT = ptp.tile([PT, QL], BF16, tag="psT")
                        for j in range(4):
                            et = g * 4 + j
                            nc.tensor.transpose(
                                psT[:, j * PT:(j + 1) * PT],
                                xnb[:, et * PT:(et + 1) * PT], ident)
                        dst = xnT[:, g * 4:(g + 1) * 4,
                                  lt * PT:(lt + 1) * PT]
                        src = psT.rearrange("p (j l) -> p j l", j=4)
                        if g == 0:
                            nc.vector.tensor_copy(dst, src)
                        else:
                            nc.scalar.copy(dst, src)

            # ======== Phase 2+3: projections pipelined w/ attention ======
            with (
                tc.tile_pool(name="work", bufs=2, space="PSUM") as wkp,
                tc.tile_pool(name="av", bufs=2, space="PSUM") as avp,
                tc.tile_pool(name="bias", bufs=4) as bp,
                tc.tile_pool(name="attn", bufs=4) as ap,
                tc.tile_pool(name="rec", bufs=2) as rcp,
                tc.tile_pool(name="oo", bufs=3) as oop,
                tc.tile_pool(name="yo", bufs=2) as yop,
                tc.tile_pool(name="s1p", bufs=3) as sp,
                tc.tile_pool(name="recd", bufs=2, space="DRAM") as rdp,
            ):
                def k_proj(ot):
                    """K^T chunk ot: both l-halves into one [128,2,512]."""
                    osl = slice(ot * PT, (ot + 1) * PT)
                    ps = wkp.tile([PT, 2, QL], F32, tag="w")
                    for nh in range(2):
                        for kc in range(4):
                            nc.tensor.matmul(
                                ps[:, nh, :],
                                wk_sb[:, 2 * kc:2 * kc + 2, osl],
                                xnT[:, 2 * kc:2 * kc + 2,
                                    nh * QL:(nh + 1) * QL],
                                start=(kc == 0),
                                stop=(kc == 3 and "k" not in pbr),
                                perf_mode=DR)
                        if "k" in pbr:
                            nc.tensor.matmul(ps[:, nh, :], pbr["k"][:, osl],
                                             ones_row[:, 0:QL],
                                             start=False, stop=True)
                    psf = ps.rearrange("p a b -> p (a b)")
                    nc.vector.tensor_scalar_mul(
                        kTzA[0:64, ot, :], psf[0:64, :], ik)
                    nc.scalar.mul(kTzB[64:128, ot, :], psf[64:128, :], ik)

                def q_proj(ot):
                    osl = slice(ot * PT, (ot + 1) * PT)
                    psq = wkp.tile([PT, 2, QL], F32, tag="w")
                    for kc in range(4):
                        nc.tensor.matmul(
                            psq[:, 0, :], wq_sb[:, 2 * kc:2 * kc + 2, osl],
                            xnT[:, 2 * kc:2 * kc + 2, 0:QL],
                            start=(kc == 0),
                            stop=(kc == 3 and "q" not in pbr),
                            perf_mode=DR)
                    if "q" in pbr:
                        nc.tensor.matmul(psq[:, 0, :], pbr["q"][:, osl],
                                         ones_row[:, 0:QL],
                                         start=False, stop=True)
                    nc.vector.tensor_scalar_mul(
                        qT[:, ot, :], psq[:, 0, :], iq)

                def v_proj(lt, vh):
                    """V rows l-chunk lt, heads vh*8..vh*8+8 -> v3 fp8."""
                    lsl = slice(lt * PT, (lt + 1) * PT)
                    vsl = slice(vh * QL, (vh + 1) * QL)
                    psv = wkp.tile([PT, 2, QL], F32, tag="w")
                    for kc in range(4):
                        nc.tensor.matmul(
                            psv[:, 0, :],
                            xnT[:, 2 * kc:2 * kc + 2, lsl],
                            wv_sb[:, 2 * kc:2 * kc + 2, vsl],
                            start=(kc == 0),
                            stop=(kc == 3 and "v" not in pbr),
                            perf_mode=DR)
                    if "v" in pbr:
                        nc.tensor.matmul(psv[:, 0, :], ones_row[:, 0:PT],
                                         pbr["v"][:, vsl],
                                         start=False, stop=True)
                    dst = v3[:, lt // 2, lt % 2, vh * 8:(vh + 1) * 8, 0:64]
                    src = psv[:, 0, :].rearrange("p (h d) -> p h d", h=8)
                    if lt % 2 == 0:
                        nc.vector.tensor_scalar_mul(dst, src, iv)
                    else:
                        nc.scalar.mul(dst, src, iv)

                def attention(t, fillers=()):
                    """Head pair t. fillers: list of closures, one popped
                    per c-iteration and emitted after the QK matmuls so the
                    PE has work while vector/scalar produce `at`."""
                    fillers = list(fillers)
                    hA = 2 * t
                    av2 = avp.tile([65, 2, QL], F32, tag="av")
                    at = None
                    for c in range(NL):
                        cpi, cpar = divmod(c, 2)
                        csl = slice(c * PT, (c + 1) * PT)
                        if cpar == 0:
                            bt = bp.tile([PT, 2, 2, QL], FP8, tag="bt")
                            nc.sync.dma_start(bt, bias_d[t, cpi])
                            at = ap.tile([PT, 2, 2, QL], FP8, tag="at")
                        ps = wkp.tile([PT, 2, QL], F32, tag="w")
                        nc.tensor.matmul(ps[:, 0, :], kTzA[:, t, csl],
                                         qT[:, t, :], start=True, stop=True)
                        nc.tensor.matmul(ps[:, 1, :], kTzB[:, t, csl],
                                         qT[:, t, :], start=True, stop=True)
                        if fillers:
                            fillers.pop(0)()
                        s1 = sp.tile([PT, 2 * QL], BF16, tag="s1")
                        nc.vector.tensor_add(
                            s1, ps.rearrange("p h q -> p (h q)"),
                            bt[:, cpar, :, :].rearrange("p h q -> p (h q)"))
                        kmb = km_sb[:, c:c + 1] if use_mask else shift_t
                        if same_gate:
                            nc.scalar.activation(
                                at[:, cpar, :, :].rearrange(
                                    "p h q -> p (h q)"),
                                s1, mybir.ActivationFunctionType.Exp,
                                bias=kmb, scale=gates[hA])
                        else:
                            s1h = s1.rearrange("p (h q) -> p h q", h=2)
                            for hi in range(2):
                                nc.scalar.activation(
                                    at[:, cpar, hi, :], s1h[:, hi, :],
                                    mybir.ActivationFunctionType.Exp,
                                    bias=kmb, scale=gates[hA + hi])
                        if cpar == 1:
                            for hi in range(2):
                                nc.tensor.matmul(
                                    av2[:, hi, :],
                                    v3[:, cpi, :, hA + hi, :],
                                    at[:, :, hi, :],
                                    start=(cpi == 0), stop=(cpi == CP - 1),
                                    perf_mode=DR)
                    # normalize: rowsum row -> DRAM roundtrip broadcast
                    # (gpsimd queue) -> approx recip -> mul
                    rec = rcp.tile([65, 2, QL], F32, tag="rec")
                    nc.vector.tensor_copy(rec[64:65, :, :],
                                          av2[64:65, :, :])
                    recd = rdp.tile([1, 2 * QL], F32, tag="recd")
                    nc.gpsimd.dma_start(
                        recd, rec[64:65, :, :].rearrange("p a b -> p (a b)"))
                    rbs = oop.tile([64, 2 * QL], F32, tag="rbs")
                    nc.gpsimd.dma_start(
                        rbs, recd[0:1, :].partition_broadcast(64))
                    nc.vector.reciprocal_approx_fast(out=rbs, in_=rbs)
                    if use_mask:
                        nc.gpsimd.tensor_mul(rbs, rbs, mqb)
                    rbs2 = rbs.rearrange("p (a b) -> p a b", a=2)
                    nc.vector.tensor_mul(oT[0:64, t, :], av2[0:64, 0, :],
                                         rbs2[:, 0, :])
                    ot_odd = oop.tile([64, QL], FP8, tag="oo")
                    nc.vector.tensor_mul(ot_odd, av2[0:64, 1, :],
                                         rbs2[:, 1, :])
                    nc.sync.dma_start(oT[64:128, t, :], ot_odd)

                for ot in range(4):
                    k_proj(ot)
                    q_proj(ot)
                for lt in range(NL):
                    v_proj(lt, 0)
                # remaining projection work, fed into the attention loops
                # one unit per c-iteration to keep the PE busy while
                # vector/scalar produce `at`
                units = []
                for ot in range(4, NE):
                    units.append(lambda ot=ot: k_proj(ot))
                    units.append(lambda ot=ot: q_proj(ot))
                for lt in range(NL):
                    units.append(lambda lt=lt: v_proj(lt, 1))
                attention(0, units[0:8])    # K4 Q4 K5 Q5 K6 Q6 K7 Q7
                attention(1, units[8:16])   # V vh=1 lt 0..7
                for t in range(2, HP):
                    attention(t)

                # ====== Phase 4: out-proj in [q, e] + residual ===========
                # final[q,e] = io/32 * sum_i oT32[i,q] * woT_s[i,e] + x[q,e]
                rescale = io / 32.0
                for qb in range(4):
                    qsl = slice(qb * PT, (qb + 1) * PT)
                    psf = wkp.tile([PT, 2, QL], F32, tag="w")
                    for eh in range(2):
                        esl = slice(eh * QL, (eh + 1) * QL)
                        for j in range(4):
                            nc.tensor.matmul(
                                psf[:, eh, :], oT[:, 2 * j:2 * j + 2, qsl],
                                wo_sb[:, 2 * j:2 * j + 2, esl],
                                start=(j == 0),
                                stop=(j == 3 and "o" not in pbr),
                                perf_mode=DR)
                        if "o" in pbr:
                            nc.tensor.matmul(psf[:, eh, :],
                                             ones_row[0:1, 0:PT],
                                             pbr["o"][:, esl],
                                             start=False, stop=True)
                    y_sb = yop.tile([PT, E], F32, tag="y")
                    nc.vector.scalar_tensor_tensor(
                        out=y_sb, in0=psf.rearrange("p a b -> p (a b)"),
                        scalar=rescale, in1=x_sb[:, qb, :],
                        op0=mybir.AluOpType.mult, op1=mybir.AluOpType.add)
                    nc.sync.dma_start(y_d[qsl, :], y_sb)
    return nc


def _fp8_scale(w):
    """Power-of-2 scale s so absmax(w*s) ~ 100 (fp8e4 max 240)."""
    am = float(np.max(np.abs(w)))
    if am == 0.0 or not np.isfinite(am):
        return 1.0
    return 2.0 ** math.floor(math.log2(100.0 / am))


def _prep_inputs(x, bias, mask, wq, bq, wk, bk, wv, bv, wo, bo, gate,
                 ln_g, ln_b):
    """Host-side folding + per-core sharding. Returns (in_maps, meta)."""
    gate = np.asarray(gate, np.float32)
    ln_g = np.asarray(ln_g, np.float32)
    ln_b = np.asarray(ln_b, np.float32)
    grep = np.repeat(gate, D)  # [E]
    safe_gate = bool(np.all(np.abs(gate) > 1e-6))
    if safe_gate:
        qscale = (SCALE / grep).astype(np.float32)
        exp_scales = [float(g) for g in gate]
    else:
        # fold gate into bias on host instead (gate ~ 0 edge case)
        qscale = np.full(E, SCALE, np.float32)
        exp_scales = [1.0] * H

    wqt = np.asarray(wq).T * ln_g[:, None] * qscale[None, :]
    wkt = np.asarray(wk).T * ln_g[:, None]
    wvt = np.asarray(wv).T * ln_g[:, None]
    wot = np.asarray(wo).T
    # fp8 scaling: weights scaled into fp8 range; inverse folded into the
    # PSUM->SBUF copies (q,k,v) or the final residual add (o). The out-proj
    # additionally sees oT at 32x natural (ones-col = 2^-5 rowsum trick).
    scales = {"q": _fp8_scale(wqt), "k": _fp8_scale(wkt),
              "v": _fp8_scale(wvt), "o": _fp8_scale(wot)}
    inv_scales = {n: 1.0 / s for n, s in scales.items()}
    wqt = (wqt * scales["q"]).astype(FP8_NP)
    wkt = (wkt * scales["k"]).astype(FP8_NP)
    wvt = (wvt * scales["v"]).astype(FP8_NP)
    wot = (wot * scales["o"]).astype(FP8_NP)
    bqe = ((np.asarray(wq) @ ln_b + np.asarray(bq)) * qscale
           * scales["q"]).astype(np.float32)
    bke = ((np.asarray(wk) @ ln_b + np.asarray(bk))
           * scales["k"]).astype(np.float32)
    bve = ((np.asarray(wv) @ ln_b + np.asarray(bv))
           * scales["v"]).astype(np.float32)
    boe = (np.asarray(bo, np.float32) * scales["o"] * 32.0)
    use_pbias = tuple(bool(np.any(b)) for b in (bqe, bke, bve, boe))

    mask = np.asarray(mask, np.int32)
    use_mask = not bool(np.all(mask == 1))

    def wfmt(w):  # [E_in, E_out] -> [128, 8, E]
        return np.ascontiguousarray(
            w.reshape(NE, PT, E).transpose(1, 0, 2))

    shared = {"wqt": wfmt(wqt), "wkt": wfmt(wkt), "wvt": wfmt(wvt),
              "wot": wfmt(wot),
              "zz": np.zeros((1, NE * L), BF_NP)}
    for name, use, b in zip("qkvo", use_pbias, (bqe, bke, bve, boe)):
        if use:
            shared[f"b{name}e"] = b.reshape(1, E).astype(BF_NP)

    x = np.asarray(x, np.float32)
    bias = np.asarray(bias, np.float32)
    in_maps = []
    for c in range(NCORES):
        b_idx, qh = divmod(c, 2)
        q0 = qh * QL
        xr = np.roll(x[b_idx], -q0, axis=0)  # query block first
        m = {}
        m.update(shared)
        m["xc"] = np.ascontiguousarray(
            xr.reshape(NL, PT, L).transpose(1, 0, 2))
        bs = bias[b_idx][:, q0:q0 + QL, :]  # [H, QL, L]
        bs = np.roll(bs, -q0, axis=2)       # roll key axis
        if not safe_gate:
            bs = bs * gate[:, None, None]
        # [H,(t,hp), q, k=(cp,cpar,p)] -> [t, cp, p, cpar, hp, q]
        b6 = bs.reshape(HP, 2, QL, CP, 2, PT).transpose(0, 3, 5, 4, 1, 2)
        m["biasc"] = np.ascontiguousarray(b6).astype(FP8_NP)
        if use_mask:
            mr = np.roll(mask[b_idx], -q0)
            kmf = (-10000.0 * (1.0 - mr.astype(np.float32))) - SHIFT
            m["kmc"] = np.ascontiguousarray(
                kmf.reshape(NL, PT).T).astype(np.float32)
            mq = mr[:QL].astype(np.float32)
            m["mqc"] = np.tile(mq, 2).reshape(1, 2 * QL)
        in_maps.append(m)
    return in_maps, (exp_scales, inv_scales, use_pbias, use_mask)


def kernel(**inputs):
    global LAST_RESULT
    in_maps, (exp_scales, inv_scales, use_pbias, use_mask) = \
        _prep_inputs(**inputs)
    nc = _build_nc(exp_scales, inv_scales, use_pbias, use_mask)
    if not nc.is_finalized():
        nc.finalize()
    res = run_bass_kernel_spmd(nc, in_maps, core_ids=list(range(NCORES)))
    LAST_RESULT = res
    out = np.empty((B, L, E), np.float32)
    for c in range(NCORES):
        b_idx, qh = divmod(c, 2)
        out[b_idx, qh * QL:(qh + 1) * QL, :] = res.results[c]["yc"]
    return out
